# revision 13
# baseline (speedup 1.0000x reference)
"""GRU kernel for Trainium2, 8 NeuronCores, data-parallel over batch.

Reference computation (per timestep, batch-major):
    z = sigmoid(x_t @ W_z + s @ R_z + B_z)
    r = sigmoid(x_t @ W_r + s @ R_r + B_r)
    h = tanh   (x_t @ W_h + (r*s) @ R_h + B_h)
    s = (1-z)*s + z*h
Returns final s: [B, H].

Shapes: B=128, T=1024, D=512, H=1024.  Sharding: batch 16 per core.

Kernel design (per core):
  Phase A: XP = x @ [W_r|W_z|W_h] + B  precomputed for all timesteps at full
           PE efficiency (M=128 tiles), stored to internal DRAM [BC*T, 3H].
  Phase B: sequential scan, state kept ONLY in transposed layout sT [H-chunks
           on partitions, batch on free]. Per step:
           - ps_{r,z,h} [16,H] PSUM seeded with XP[t] via identity matmuls
           - r/z recurrent matmuls in fp8 (e4m3) DoubleRow perf mode: both
             R_{r,z} (resident, pre-quantized on host) and the state sT8
             (quantized each step) are fp8; 2 K-chunks per instruction.
           - h recurrent matmul in f32r with stationary rsT = rT (.) sT.
           - sigmoid/tanh on ScalarE (batch-major), gates transposed back via
             PE transposes; state update runs entirely in transposed layout
             on [128,128] tiles: sT += zT (.) (hT - sT); sT8 = fp8(sT).
           fp8 z/r numerics: rel err 5.8e-3 vs f32 reference (validated in
           numpy emulation over the full 1024-step recurrence).
"""

import numpy as np

import concourse.bass as bass
from concourse import bacc
import concourse.mybir as mybir
from concourse.tile import TileContext
from concourse.bass_utils import run_bass_kernel_spmd
from concourse.masks import make_identity

B, T, D, H = 128, 1024, 512, 1024
NCORES = 8
BC = B // NCORES          # 16 batch rows per core
H3 = 3 * H                # gates concatenated [r|z|h]
KD = D // 128             # 4 k-chunks over input features
KH = H // 128             # 8 k-chunks over hidden dim
FP = mybir.dt.float32
FPR = mybir.dt.float32r
F8 = mybir.dt.float8e4
AF = mybir.ActivationFunctionType
OP = mybir.AluOpType
BF = mybir.dt.bfloat16
DR = mybir.MatmulPerfMode.DoubleRow
USE_DR_ZR = True     # fp8 DoubleRow for z/r gates
USE_DR_H = True      # fp8 DoubleRow (+residual) for h gate
USE_ZT_DMA = False   # DMA XBAR z-transpose races on HW; use PE


def _r(ap):
    # Bitcast an f32 AP to float32r (fast PE streaming, 1 cycle/row at N>=256)
    return ap.bitcast(FPR)


def build_gru(t_steps=T):
    nc = bacc.Bacc()
    xT = nc.declare_dram_parameter("xT", [D, BC * t_steps], FPR, False)
    Wc = nc.declare_dram_parameter("Wcat", [D, H3], FPR, False)
    Bc = nc.declare_dram_parameter("Bcat", [128, H3], FP, False)
    Rc = nc.declare_dram_parameter("Rcat", [H, H3], FPR, False)
    R8 = nc.declare_dram_parameter("R8", [H, 4 * H], F8, False)
    out = nc.declare_dram_parameter("out", [BC, H], FP, True)
    XP = nc.dram_tensor("XP", [BC * t_steps, H3], FPR)

    MT = (BC * t_steps) // 128   # number of 128-row tiles of [bt, .]
    NT = H3 // 512               # 6 n-tiles of 512
    xp3 = XP[:].rearrange("(b t) n -> t b n", b=BC)   # [t_steps, BC, H3]

    with TileContext(nc) as tc:
        with tc.tile_pool(name="const_pool", bufs=1) as cp:
            ident_t = cp.tile([16, 16], FP)
            make_identity(nc, ident_t[:])
            ident = cp.tile([16, 16], FPR)
            nc.scalar.copy(out=ident[:], in_=ident_t[:])
            ident128 = cp.tile([128, 128], FP)
            make_identity(nc, ident128[:])

            # ---------------- phase A: XP = x @ Wcat + B ----------------
            with (
                tc.tile_pool(name="phase_a_w", bufs=1) as wp,
                tc.tile_pool(name="a_x", bufs=4) as axp,
                tc.tile_pool(name="a_ps", bufs=4, space="PSUM") as aps,
                tc.tile_pool(name="a_out", bufs=4) as aop,
            ):
                # bias arrives pre-broadcast over 128 partitions from the host
                bias_bc = wp.tile([128, H3], FP)
                nc.sync.dma_start(out=bias_bc[:], in_=Bc[:, :])

                w_sb = wp.tile([128, KD * H3], FPR)
                nc.sync.dma_start(
                    out=w_sb[:],
                    in_=Wc[:].rearrange("(kd p) n -> p kd n", kd=KD),
                )
                xT_v = xT[:].rearrange("(kd p) m -> p kd m", kd=KD)
                for mt in range(MT):
                    x_sb = axp.tile([128, KD * 128], FPR)
                    nc.sync.dma_start(
                        out=x_sb[:],
                        in_=xT_v[:, :, mt * 128:(mt + 1) * 128],
                    )
                    for ntile in range(NT):
                        ps = aps.tile([128, 512], FP, tag="a_ps")
                        for kd in range(KD):
                            nc.tensor.matmul(
                                ps[:],
                                x_sb[:, kd * 128:(kd + 1) * 128],
                                w_sb[:, kd * H3 + ntile * 512: kd * H3 + (ntile + 1) * 512],
                                start=(kd == 0),
                                stop=(kd == KD - 1),
                            )
                        o_sb = aop.tile([128, 512], FPR)
                        nc.vector.tensor_tensor(
                            o_sb[:], ps[:], bias_bc[:, ntile * 512:(ntile + 1) * 512],
                            OP.add,
                        )
                        nc.sync.dma_start(
                            out=XP[mt * 128:(mt + 1) * 128,
                                   ntile * 512:(ntile + 1) * 512],
                            in_=o_sb[:],
                        )

            # ---------------- phase B: the scan ----------------
            with (
                tc.tile_pool(name="scan_state", bufs=1) as stp,
                tc.tile_pool(name="xp_in", bufs=3) as xpp,
                tc.tile_pool(name="gate_ps", bufs=1, space="PSUM") as gpp,
                tc.tile_pool(name="tr_ps", bufs=2, space="PSUM") as trp,
                tc.tile_pool(name="ew", bufs=2) as ewp,
            ):
                scan_body(nc, tc, stp, xpp, gpp, trp, ewp, ident, ident_t,
                          ident128, Rc, R8, XP, xp3, out, t_steps)
    nc.finalize()
    return nc


def scan_body(nc, tc, stp, xpp, gpp, trp, ewp, ident, ident_t, ident128,
              Rc, R8, XP, xp3, out, t_steps):
    # resident recurrent weights, all fp8: [Rr8|Rz8|Rh8|dRh8] (32KB/part)
    R8_sb = stp.tile([128, KH * 4 * H], F8)
    nc.sync.dma_start(
        out=R8_sb[:],
        in_=R8[:].rearrange("(kh p) n -> p kh n", kh=KH),
    )
    R8_v = R8_sb[:].rearrange("p (kh n) -> p kh n", kh=KH)
    Rf_v = None
    if not (USE_DR_ZR and USE_DR_H):
        Rf_sb = stp.tile([128, KH * H3], FPR)
        nc.sync.dma_start(
            out=Rf_sb[:],
            in_=Rc[:].rearrange("(kh p) n -> p kh n", kh=KH),
        )
        Rf_v = Rf_sb[:].rearrange("p (kh n) -> p kh n", kh=KH)

    # State lives ONLY transposed, split in two half tiles (kh 0-3 / 4-7) so
    # every consumer dependency is half-granular: the tail of step t overlaps
    # the head of step t+1.
    KH2 = KH // 2
    HW_ = KH2 * 16                      # 64 free elements per half
    sT = [stp.tile([128, HW_], FP, name=f"sT{i}") for i in range(2)]
    sT8 = [stp.tile([128, HW_], F8, name=f"sT8_{i}") for i in range(2)]
    for hf in range(2):
        nc.gpsimd.memset(sT[hf][:], 0.0)
        nc.gpsimd.memset(sT8[hf][:].bitcast(FP), 0.0)

    def sT8_pair(kp):
        # stationary AP [128, 2, 16] for k-chunk pair (2kp, 2kp+1)
        hf, j = divmod(kp, 2)
        return sT8[hf][:].rearrange("p (k m) -> p k m", k=KH2)[:, 2 * j:2 * j + 2, :]

    def inject(ps, xp, gate, ntile):
        lo = ntile * 512
        nc.tensor.matmul(
            ps[:], ident[:], xp[:, gate * H + lo: gate * H + lo + 512],
            start=True, stop=False,
        )

    def dr_matmuls(ps, gate, ntile):
        """ps[16,512] += sT8.T @ R8[:, gate-ntile] via fp8 DoubleRow."""
        lo = gate * H + ntile * 512
        if USE_DR_ZR:
            for kp in range(KH // 2):
                nc.tensor.matmul(
                    ps[:], sT8_pair(kp),
                    R8_v[:, 2 * kp:2 * kp + 2, lo:lo + 512],
                    start=False, stop=(kp == KH // 2 - 1),
                    perf_mode=DR,
                )
        else:
            for kh in range(KH):
                hf, j = divmod(kh, KH2)
                nc.tensor.matmul(
                    ps[:], _r(sT[hf][:, j * 16:j * 16 + 16]),
                    Rf_v[:, kh, lo:lo + 512],
                    start=False, stop=(kh == KH - 1),
                )

    def h_matmuls(ps, rs8, ntile):
        if USE_DR_H:
            # ps += rs8.T @ (Rh8 + dRh8), both fp8 DoubleRow passes
            for blk in (2, 3):
                lo = blk * H + ntile * 512
                for kp in range(KH // 2):
                    hf, j = divmod(kp, 2)
                    pair = rs8[hf][:].rearrange("p (k m) -> p k m", k=KH2)[:, 2 * j:2 * j + 2, :]
                    nc.tensor.matmul(
                        ps[:], pair,
                        R8_v[:, 2 * kp:2 * kp + 2, lo:lo + 512],
                        start=False, stop=(blk == 3 and kp == KH // 2 - 1),
                        perf_mode=DR,
                    )
        else:
            lo = 2 * H + ntile * 512
            for kh in range(KH):
                hf, j = divmod(kh, KH2)
                nc.tensor.matmul(
                    ps[:], rs8[hf][:, j * 16:j * 16 + 16],
                    Rf_v[:, kh, lo:lo + 512],
                    start=False, stop=(kh == KH - 1),
                )

    def transpose_half(dst_ps, src_sb, hf):
        # 4 chunk transposes of src half hf into a [128, 64] psum tile
        for j in range(KH2):
            kh = hf * KH2 + j
            nc.tensor.transpose(
                dst_ps[:, j * 16:(j + 1) * 16],
                src_sb[:, kh * 128:(kh + 1) * 128], ident_t[:]
            )

    def alloc_ps(t):
        _ = t
        return {g: [gpp.tile([16, 512], FP, tag=f"ps_{g}{b}", name=f"ps_{g}{b}_{t}")
                    for b in range(2)]
                for g in "rzh"}

    def alloc_xp(t):
        xp = xpp.tile([16, H3], FPR, tag="xp")
        nc.sync.dma_start(out=xp[:], in_=xp3[t])
        return xp

    xp = alloc_xp(0)
    ps = alloc_ps(0)
    for g, gi in (("r", 0), ("z", 1), ("h", 2)):
        for b in range(2):
            inject(ps[g][b], xp, gi, b)

    for t in range(t_steps):
        last = t + 1 >= t_steps
        # ---- r/z gates: fp8 DR matmuls ----
        for b in range(2):
            dr_matmuls(ps["r"][b], 0, b)
        for b in range(2):
            dr_matmuls(ps["z"][b], 1, b)
        if not last:
            nxp = alloc_xp(t + 1)
            nps = alloc_ps(t + 1)

        r_sb = ewp.tile([16, H], FP, tag="r")
        nc.scalar.activation(r_sb[:, 0:512], ps["r"][0][:], AF.Sigmoid)
        nc.scalar.activation(r_sb[:, 512:], ps["r"][1][:], AF.Sigmoid)

        # transpose psum tiles share one 2-buf rotation (2 banks total); the
        # z transposes are emitted LAST so their bank reuse never cycles.
        tps_r = [trp.tile([128, HW_], FP, tag="tr", name=f"tps_r{hf}_{t}")
                 for hf in range(2)]
        rs8 = [ewp.tile([128, HW_], F8 if USE_DR_H else FPR,
                        tag=f"rs8{hf}", name=f"rs8{hf}_{t}")
               for hf in range(2)]
        for hf in range(2):
            transpose_half(tps_r[hf], r_sb, hf)
        if not last:     # PE filler between r-transposes and h-matmuls
            for b in range(2):
                inject(nps["r"][b], nxp, 0, b)
        for hf in range(2):
            nc.vector.tensor_tensor(rs8[hf][:], tps_r[hf][:], sT[hf][:],
                                    OP.mult)

        # ---- h gate: fp8 DR matmuls with stationary rs8 ----
        h_matmuls(ps["h"][0], rs8, 0)
        h_matmuls(ps["h"][1], rs8, 1)

        # z in bf16; transposed via the DMA XBAR (off the PE/dep chain)
        z_sb = ewp.tile([16, H], BF if USE_ZT_DMA else FP, tag="z")
        if USE_ZT_DMA:
            zT = ewp.tile([128, KH * 16], BF, tag="zT")
            # one half-DMA per sigma half: writer->reader dep stays 1:1
            nc.scalar.activation(z_sb[:, 0:512], ps["z"][0][:], AF.Sigmoid)
            nc.sync.dma_start_transpose(
                out=zT[:, :HW_].rearrange("p (c m) -> p c m", c=KH2),
                in_=z_sb[:, 0:512])
            nc.scalar.activation(z_sb[:, 512:], ps["z"][1][:], AF.Sigmoid)
            nc.sync.dma_start_transpose(
                out=zT[:, HW_:].rearrange("p (c m) -> p c m", c=KH2),
                in_=z_sb[:, 512:])
        else:
            nc.scalar.activation(z_sb[:, 0:512], ps["z"][0][:], AF.Sigmoid)
            nc.scalar.activation(z_sb[:, 512:], ps["z"][1][:], AF.Sigmoid)
        if not last:
            for b in range(2):
                inject(nps["z"][b], nxp, 1, b)

        # ---- tanh + h transposes; z transposes after (bank rotation) ----
        h_sb = ewp.tile([16, H], FP, tag="h")
        tps_h = [trp.tile([128, HW_], FP, tag="tr", name=f"tps_h{hf}_{t}")
                 for hf in range(2)]
        for hf in range(2):
            sl = slice(hf * 512, (hf + 1) * 512)
            nc.scalar.activation(h_sb[:, sl], ps["h"][hf][:], AF.Tanh)
            transpose_half(tps_h[hf], h_sb, hf)
        if USE_ZT_DMA:
            zT_src = [zT[:, hf * HW_:(hf + 1) * HW_] for hf in range(2)]
        else:
            tps_zpe = [trp.tile([128, HW_], FP, tag="tr", name=f"tps_z{hf}_{t}")
                       for hf in range(2)]
            for hf in range(2):
                transpose_half(tps_zpe[hf], z_sb, hf)
            zT_src = [tps_zpe[hf][:] for hf in range(2)]
        if not last:
            for hf in range(2):
                inject(nps["h"][hf], nxp, 2, hf)

        # ---- transposed update: sT += zT * (hT - sT);  sT8 = fp8(sT) ----
        for hf in range(2):
            dT = ewp.tile([128, HW_], FP, tag=f"dT{hf}", name=f"dT{hf}_{t}")
            nc.vector.tensor_tensor(dT[:], tps_h[hf][:], sT[hf][:], OP.subtract)
            zdT = ewp.tile([128, HW_], FP, tag=f"zdT{hf}", name=f"zdT{hf}_{t}")
            nc.vector.tensor_tensor(zdT[:], zT_src[hf], dT[:], OP.mult)
            nc.vector.tensor_tensor(sT8[hf][:], sT[hf][:], zdT[:], OP.add)
            nc.vector.tensor_tensor(_r(sT[hf][:]), sT[hf][:], zdT[:], OP.add)

        if not last:
            xp, ps = nxp, nps

    # transpose the final state back to batch-major and store
    po = [gpp.tile([16, 512], FP, tag=f"ps_r{b}", name=f"po{b}") for b in range(2)]
    for kh in range(KH):
        hf, j = divmod(kh, KH2)
        b, jj = divmod(kh, KH2)
        nc.tensor.transpose(
            po[b][:, jj * 128:(jj + 1) * 128],
            sT[hf][:, j * 16:(j + 1) * 16], ident128[:]
        )
    s_out = ewp.tile([16, H], FP, tag="r")
    nc.scalar.copy(out=s_out[:, 0:512], in_=po[0][:])
    nc.scalar.copy(out=s_out[:, 512:], in_=po[1][:])
    nc.sync.dma_start(out=out[:, :], in_=s_out[:])


_CACHE = {}


def _get_nc(t_steps=T):
    key = t_steps
    if key not in _CACHE:
        _CACHE[key] = build_gru(t_steps)
    return _CACHE[key]


def prepare_in_maps(inputs, t_steps=T):
    f8np = mybir.dt.np(F8)
    x = np.asarray(inputs["x"], dtype=np.float32)
    Wcat = np.ascontiguousarray(
        np.concatenate([np.asarray(inputs["W_r"]), np.asarray(inputs["W_z"]),
                        np.asarray(inputs["W_h"])], axis=1),
        dtype=np.float32,
    )
    Rcat = np.ascontiguousarray(
        np.concatenate([np.asarray(inputs["R_r"]), np.asarray(inputs["R_z"]),
                        np.asarray(inputs["R_h"])], axis=1),
        dtype=np.float32,
    )
    Rh8 = Rcat[:, 2 * H:].astype(f8np)
    dRh8 = (Rcat[:, 2 * H:] - Rh8.astype(np.float32)).astype(f8np)
    R8 = np.ascontiguousarray(
        np.concatenate([Rcat[:, :2 * H].astype(f8np), Rh8, dRh8], axis=1))
    Bcat = np.ascontiguousarray(
        np.broadcast_to(
            np.concatenate([np.asarray(inputs["B_r"]), np.asarray(inputs["B_z"]),
                            np.asarray(inputs["B_h"])])[None, :], (128, H3)),
        dtype=np.float32,
    )
    in_maps = []
    for c in range(NCORES):
        xc = x[c * BC:(c + 1) * BC, :t_steps, :]          # [BC, t, D]
        xTc = np.ascontiguousarray(
            xc.transpose(2, 0, 1).reshape(D, BC * t_steps)
        )
        in_maps.append({"xT": xTc, "Wcat": Wcat, "Bcat": Bcat, "Rcat": Rcat,
                        "R8": R8})
    return in_maps


def gather_outputs(per_core_results):
    outs = [per_core_results[c]["out"] for c in range(NCORES)]
    return np.concatenate(outs, axis=0)


def kernel_run(x, W_z, W_r, W_h, R_z, R_r, R_h, B_z, B_r, B_h, t_steps=T, **run_kw):
    inputs = dict(x=x, W_z=W_z, W_r=W_r, W_h=W_h, R_z=R_z, R_r=R_r, R_h=R_h,
                  B_z=B_z, B_r=B_r, B_h=B_h)
    in_maps = prepare_in_maps(inputs, t_steps)
    res = run_bass_kernel_spmd(_get_nc(t_steps), in_maps, list(range(NCORES)), **run_kw)
    full = gather_outputs(res.results)
    return full, res


def kernel(**inputs):
    full, _ = kernel_run(**inputs)
    return full


# revision 15
# speedup vs baseline: 1.0425x; 1.0425x over previous
"""GRU kernel for Trainium2, 8 NeuronCores, data-parallel over batch.

Reference computation (per timestep, batch-major):
    z = sigmoid(x_t @ W_z + s @ R_z + B_z)
    r = sigmoid(x_t @ W_r + s @ R_r + B_r)
    h = tanh   (x_t @ W_h + (r*s) @ R_h + B_h)
    s = (1-z)*s + z*h
Returns final s: [B, H].

Shapes: B=128, T=1024, D=512, H=1024.  Sharding: batch 16 per core.

Kernel design (per core):
  Phase A: XP = x @ [W_r|W_z|W_h] + B  precomputed for all timesteps at full
           PE efficiency (M=128 tiles), stored to internal DRAM [BC*T, 3H].
  Phase B: sequential scan, state kept ONLY in transposed layout sT [H-chunks
           on partitions, batch on free], split in per-PSUM-bank halves so all
           dependencies are half-granular. Per step:
           - ps_{r,z,h} 2x[16,512] PSUM banks seeded with XP[t] via f32r
             identity matmuls (next step's injects run as PE fillers).
           - ALL recurrent matmuls in fp8 (e4m3) DoubleRow perf mode (2
             K-chunks per instruction, 0.5 cycles/row): z/r use the fp8 state
             sT8; h uses rs8 = fp8(rT (.) sT) with the weight residual
             R_h ~ Rh8 + dRh8 (two DR passes) compensating fp8 quantization.
           - sigmoid/tanh on ScalarE (batch-major); r/z/h transposed back via
             PE transposes (z early, evacuated to SBUF mid-step); transposed
             update sT8 = fp8(sT + zT (.) (hT - sT)) fused on VectorE.
           Numerics (numpy emulation, full 1024 steps): rel err 1.26e-2 vs
           f32 reference; HW measured 1.257e-2 (< 2e-2 gate).
"""

import numpy as np

import concourse.bass as bass
from concourse import bacc
import concourse.mybir as mybir
from concourse.tile import TileContext
from concourse.bass_utils import run_bass_kernel_spmd
from concourse.masks import make_identity

B, T, D, H = 128, 1024, 512, 1024
NCORES = 8
BC = B // NCORES          # 16 batch rows per core
H3 = 3 * H                # gates concatenated [r|z|h]
KD = D // 128             # 4 k-chunks over input features
KH = H // 128             # 8 k-chunks over hidden dim
FP = mybir.dt.float32
FPR = mybir.dt.float32r
F8 = mybir.dt.float8e4
AF = mybir.ActivationFunctionType
OP = mybir.AluOpType
BF = mybir.dt.bfloat16
DR = mybir.MatmulPerfMode.DoubleRow
USE_DR_ZR = True     # fp8 DoubleRow for z/r gates
USE_DR_H = True      # fp8 DoubleRow (+residual) for h gate
USE_ZT_DMA = False   # DMA XBAR z-transpose races on HW; use PE


def _r(ap):
    # Bitcast an f32 AP to float32r (fast PE streaming, 1 cycle/row at N>=256)
    return ap.bitcast(FPR)


def build_gru(t_steps=T):
    nc = bacc.Bacc()
    xT = nc.declare_dram_parameter("xT", [D, BC * t_steps], FPR, False)
    Wc = nc.declare_dram_parameter("Wcat", [D, H3], FPR, False)
    Bc = nc.declare_dram_parameter("Bcat", [128, H3], FP, False)
    Rc = nc.declare_dram_parameter("Rcat", [H, H3], FPR, False)
    R8 = nc.declare_dram_parameter("R8", [H, 4 * H], F8, False)
    out = nc.declare_dram_parameter("out", [BC, H], FP, True)
    XP = nc.dram_tensor("XP", [BC * t_steps, H3], FPR)

    MT = (BC * t_steps) // 128   # number of 128-row tiles of [bt, .]
    NT = H3 // 512               # 6 n-tiles of 512
    xp3 = XP[:].rearrange("(b t) n -> t b n", b=BC)   # [t_steps, BC, H3]

    with TileContext(nc) as tc:
        with tc.tile_pool(name="const_pool", bufs=1) as cp:
            ident_t = cp.tile([16, 16], FP)
            make_identity(nc, ident_t[:])
            ident = cp.tile([16, 16], FPR)
            nc.scalar.copy(out=ident[:], in_=ident_t[:])
            ident128 = cp.tile([128, 128], FP)
            make_identity(nc, ident128[:])

            # ---------------- phase A: XP = x @ Wcat + B ----------------
            with (
                tc.tile_pool(name="phase_a_w", bufs=1) as wp,
                tc.tile_pool(name="a_x", bufs=4) as axp,
                tc.tile_pool(name="a_ps", bufs=4, space="PSUM") as aps,
                tc.tile_pool(name="a_out", bufs=4) as aop,
            ):
                # bias arrives pre-broadcast over 128 partitions from the host
                bias_bc = wp.tile([128, H3], FP)
                nc.sync.dma_start(out=bias_bc[:], in_=Bc[:, :])

                w_sb = wp.tile([128, KD * H3], FPR)
                nc.sync.dma_start(
                    out=w_sb[:],
                    in_=Wc[:].rearrange("(kd p) n -> p kd n", kd=KD),
                )
                xT_v = xT[:].rearrange("(kd p) m -> p kd m", kd=KD)
                for mt in range(MT):
                    x_sb = axp.tile([128, KD * 128], FPR)
                    nc.sync.dma_start(
                        out=x_sb[:],
                        in_=xT_v[:, :, mt * 128:(mt + 1) * 128],
                    )
                    for ntile in range(NT):
                        ps = aps.tile([128, 512], FP, tag="a_ps")
                        for kd in range(KD):
                            nc.tensor.matmul(
                                ps[:],
                                x_sb[:, kd * 128:(kd + 1) * 128],
                                w_sb[:, kd * H3 + ntile * 512: kd * H3 + (ntile + 1) * 512],
                                start=(kd == 0),
                                stop=(kd == KD - 1),
                            )
                        o_sb = aop.tile([128, 512], FPR)
                        nc.vector.tensor_tensor(
                            o_sb[:], ps[:], bias_bc[:, ntile * 512:(ntile + 1) * 512],
                            OP.add,
                        )
                        nc.sync.dma_start(
                            out=XP[mt * 128:(mt + 1) * 128,
                                   ntile * 512:(ntile + 1) * 512],
                            in_=o_sb[:],
                        )

            # ---------------- phase B: the scan ----------------
            with (
                tc.tile_pool(name="scan_state", bufs=1) as stp,
                tc.tile_pool(name="xp_in", bufs=3) as xpp,
                tc.tile_pool(name="gate_ps", bufs=1, space="PSUM") as gpp,
                tc.tile_pool(name="tr_ps", bufs=2, space="PSUM") as trp,
                tc.tile_pool(name="ew", bufs=2) as ewp,
            ):
                scan_body(nc, tc, stp, xpp, gpp, trp, ewp, ident, ident_t,
                          ident128, Rc, R8, XP, xp3, out, t_steps)
    nc.finalize()
    return nc


def scan_body(nc, tc, stp, xpp, gpp, trp, ewp, ident, ident_t, ident128,
              Rc, R8, XP, xp3, out, t_steps):
    # resident recurrent weights, all fp8: [Rr8|Rz8|Rh8|dRh8] (32KB/part)
    R8_sb = stp.tile([128, KH * 4 * H], F8)
    nc.sync.dma_start(
        out=R8_sb[:],
        in_=R8[:].rearrange("(kh p) n -> p kh n", kh=KH),
    )
    R8_v = R8_sb[:].rearrange("p (kh n) -> p kh n", kh=KH)
    Rf_v = None
    if not (USE_DR_ZR and USE_DR_H):
        Rf_sb = stp.tile([128, KH * H3], FPR)
        nc.sync.dma_start(
            out=Rf_sb[:],
            in_=Rc[:].rearrange("(kh p) n -> p kh n", kh=KH),
        )
        Rf_v = Rf_sb[:].rearrange("p (kh n) -> p kh n", kh=KH)

    # State lives ONLY transposed, split in two half tiles (kh 0-3 / 4-7) so
    # every consumer dependency is half-granular: the tail of step t overlaps
    # the head of step t+1.
    KH2 = KH // 2
    HW_ = KH2 * 16                      # 64 free elements per half
    sT = [stp.tile([128, HW_], FP, name=f"sT{i}") for i in range(2)]
    sT8 = [stp.tile([128, HW_], F8, name=f"sT8_{i}") for i in range(2)]
    for hf in range(2):
        nc.gpsimd.memset(sT[hf][:], 0.0)
        nc.gpsimd.memset(sT8[hf][:].bitcast(FP), 0.0)

    def sT8_pair(kp):
        # stationary AP [128, 2, 16] for k-chunk pair (2kp, 2kp+1)
        hf, j = divmod(kp, 2)
        return sT8[hf][:].rearrange("p (k m) -> p k m", k=KH2)[:, 2 * j:2 * j + 2, :]

    def inject(ps, xp, gate, ntile):
        lo = ntile * 512
        nc.tensor.matmul(
            ps[:], ident[:], xp[:, gate * H + lo: gate * H + lo + 512],
            start=True, stop=False,
        )

    def dr_matmuls(ps, gate, ntile):
        """ps[16,512] += sT8.T @ R8[:, gate-ntile] via fp8 DoubleRow."""
        lo = gate * H + ntile * 512
        if USE_DR_ZR:
            for kp in range(KH // 2):
                nc.tensor.matmul(
                    ps[:], sT8_pair(kp),
                    R8_v[:, 2 * kp:2 * kp + 2, lo:lo + 512],
                    start=False, stop=(kp == KH // 2 - 1),
                    perf_mode=DR,
                )
        else:
            for kh in range(KH):
                hf, j = divmod(kh, KH2)
                nc.tensor.matmul(
                    ps[:], _r(sT[hf][:, j * 16:j * 16 + 16]),
                    Rf_v[:, kh, lo:lo + 512],
                    start=False, stop=(kh == KH - 1),
                )

    def h_matmuls(ps, rs8, ntile):
        if USE_DR_H:
            # ps += rs8.T @ (Rh8 + dRh8), both fp8 DoubleRow passes
            for blk in (2, 3):
                lo = blk * H + ntile * 512
                for kp in range(KH // 2):
                    hf, j = divmod(kp, 2)
                    pair = rs8[hf][:].rearrange("p (k m) -> p k m", k=KH2)[:, 2 * j:2 * j + 2, :]
                    nc.tensor.matmul(
                        ps[:], pair,
                        R8_v[:, 2 * kp:2 * kp + 2, lo:lo + 512],
                        start=False, stop=(blk == 3 and kp == KH // 2 - 1),
                        perf_mode=DR,
                    )
        else:
            lo = 2 * H + ntile * 512
            for kh in range(KH):
                hf, j = divmod(kh, KH2)
                nc.tensor.matmul(
                    ps[:], rs8[hf][:, j * 16:j * 16 + 16],
                    Rf_v[:, kh, lo:lo + 512],
                    start=False, stop=(kh == KH - 1),
                )

    def transpose_half(dst_ps, src_sb, hf):
        # 4 chunk transposes of src half hf into a [128, 64] psum tile
        for j in range(KH2):
            kh = hf * KH2 + j
            nc.tensor.transpose(
                dst_ps[:, j * 16:(j + 1) * 16],
                src_sb[:, kh * 128:(kh + 1) * 128], ident_t[:]
            )

    def alloc_ps(t):
        _ = t
        return {g: [gpp.tile([16, 512], FP, tag=f"ps_{g}{b}", name=f"ps_{g}{b}_{t}")
                    for b in range(2)]
                for g in "rzh"}

    def alloc_xp(t):
        xp = xpp.tile([16, H3], FPR, tag="xp")
        nc.sync.dma_start(out=xp[:], in_=xp3[t])
        return xp

    xp = alloc_xp(0)
    ps = alloc_ps(0)
    for g, gi in (("r", 0), ("z", 1), ("h", 2)):
        for b in range(2):
            inject(ps[g][b], xp, gi, b)

    for t in range(t_steps):
        last = t + 1 >= t_steps
        # ---- r/z gates: fp8 DR matmuls ----
        for b in range(2):
            dr_matmuls(ps["r"][b], 0, b)
        for b in range(2):
            dr_matmuls(ps["z"][b], 1, b)
        if not last:
            nxp = alloc_xp(t + 1)
            nps = alloc_ps(t + 1)

        r_sb = ewp.tile([16, H], FP, tag="r")
        nc.scalar.activation(r_sb[:, 0:512], ps["r"][0][:], AF.Sigmoid)
        nc.scalar.activation(r_sb[:, 512:], ps["r"][1][:], AF.Sigmoid)

        # transpose psum tiles share one 2-buf rotation (2 banks total); the
        # z transposes are emitted LAST so their bank reuse never cycles.
        tps_r = [trp.tile([128, HW_], FP, tag="tr", name=f"tps_r{hf}_{t}")
                 for hf in range(2)]
        rs8 = [ewp.tile([128, HW_], F8 if USE_DR_H else FPR,
                        tag=f"rs8{hf}", name=f"rs8{hf}_{t}")
               for hf in range(2)]
        for hf in range(2):
            transpose_half(tps_r[hf], r_sb, hf)
        if not last:     # PE filler between r-transposes and h-matmuls
            for b in range(2):
                inject(nps["r"][b], nxp, 0, b)
        for hf in range(2):
            nc.vector.tensor_tensor(rs8[hf][:], tps_r[hf][:], sT[hf][:],
                                    OP.mult)

        # ---- h gate: fp8 DR matmuls with stationary rs8 ----
        h_matmuls(ps["h"][0], rs8, 0)
        h_matmuls(ps["h"][1], rs8, 1)

        # z: sigmoid, PE-transpose mid-step (PE bubbles during h matmuls),
        # then evacuate to SBUF early so the tail never waits on PE for z.
        z_sb = ewp.tile([16, H], FP, tag="z")
        nc.scalar.activation(z_sb[:, 0:512], ps["z"][0][:], AF.Sigmoid)
        nc.scalar.activation(z_sb[:, 512:], ps["z"][1][:], AF.Sigmoid)
        tps_z = [trp.tile([128, HW_], FP, tag="tr", name=f"tps_z{hf}_{t}")
                 for hf in range(2)]
        zTs = [ewp.tile([128, HW_], FP, tag=f"zTs{hf}", name=f"zTs{hf}_{t}")
               for hf in range(2)]
        for hf in range(2):
            transpose_half(tps_z[hf], z_sb, hf)
            nc.vector.tensor_copy(zTs[hf][:], tps_z[hf][:])
        zT_src = [zTs[hf][:] for hf in range(2)]
        if not last:
            for b in range(2):
                inject(nps["z"][b], nxp, 1, b)

        # ---- tanh + h transposes ----
        h_sb = ewp.tile([16, H], FP, tag="h")
        tps_h = [trp.tile([128, HW_], FP, tag="tr", name=f"tps_h{hf}_{t}")
                 for hf in range(2)]
        for hf in range(2):
            sl = slice(hf * 512, (hf + 1) * 512)
            nc.scalar.activation(h_sb[:, sl], ps["h"][hf][:], AF.Tanh)
            transpose_half(tps_h[hf], h_sb, hf)
        if not last:
            for hf in range(2):
                inject(nps["h"][hf], nxp, 2, hf)

        # ---- transposed update: sT += zT * (hT - sT);  sT8 = fp8(sT) ----
        for hf in range(2):
            dT = ewp.tile([128, HW_], FP, tag=f"dT{hf}", name=f"dT{hf}_{t}")
            nc.vector.tensor_tensor(dT[:], tps_h[hf][:], sT[hf][:], OP.subtract)
            zdT = ewp.tile([128, HW_], FP, tag=f"zdT{hf}", name=f"zdT{hf}_{t}")
            nc.vector.tensor_tensor(zdT[:], zT_src[hf], dT[:], OP.mult)
            nc.vector.tensor_tensor(sT8[hf][:], sT[hf][:], zdT[:], OP.add)
            nc.vector.tensor_tensor(_r(sT[hf][:]), sT[hf][:], zdT[:], OP.add)

        if not last:
            xp, ps = nxp, nps

    # transpose the final state back to batch-major and store
    po = [gpp.tile([16, 512], FP, tag=f"ps_r{b}", name=f"po{b}") for b in range(2)]
    for kh in range(KH):
        hf, j = divmod(kh, KH2)
        b, jj = divmod(kh, KH2)
        nc.tensor.transpose(
            po[b][:, jj * 128:(jj + 1) * 128],
            sT[hf][:, j * 16:(j + 1) * 16], ident128[:]
        )
    s_out = ewp.tile([16, H], FP, tag="r")
    nc.scalar.copy(out=s_out[:, 0:512], in_=po[0][:])
    nc.scalar.copy(out=s_out[:, 512:], in_=po[1][:])
    nc.sync.dma_start(out=out[:, :], in_=s_out[:])


_CACHE = {}


def _get_nc(t_steps=T):
    key = t_steps
    if key not in _CACHE:
        _CACHE[key] = build_gru(t_steps)
    return _CACHE[key]


def prepare_in_maps(inputs, t_steps=T):
    f8np = mybir.dt.np(F8)
    x = np.asarray(inputs["x"], dtype=np.float32)
    Wcat = np.ascontiguousarray(
        np.concatenate([np.asarray(inputs["W_r"]), np.asarray(inputs["W_z"]),
                        np.asarray(inputs["W_h"])], axis=1),
        dtype=np.float32,
    )
    Rcat = np.ascontiguousarray(
        np.concatenate([np.asarray(inputs["R_r"]), np.asarray(inputs["R_z"]),
                        np.asarray(inputs["R_h"])], axis=1),
        dtype=np.float32,
    )
    Rh8 = Rcat[:, 2 * H:].astype(f8np)
    dRh8 = (Rcat[:, 2 * H:] - Rh8.astype(np.float32)).astype(f8np)
    R8 = np.ascontiguousarray(
        np.concatenate([Rcat[:, :2 * H].astype(f8np), Rh8, dRh8], axis=1))
    Bcat = np.ascontiguousarray(
        np.broadcast_to(
            np.concatenate([np.asarray(inputs["B_r"]), np.asarray(inputs["B_z"]),
                            np.asarray(inputs["B_h"])])[None, :], (128, H3)),
        dtype=np.float32,
    )
    in_maps = []
    for c in range(NCORES):
        xc = x[c * BC:(c + 1) * BC, :t_steps, :]          # [BC, t, D]
        xTc = np.ascontiguousarray(
            xc.transpose(2, 0, 1).reshape(D, BC * t_steps)
        )
        in_maps.append({"xT": xTc, "Wcat": Wcat, "Bcat": Bcat, "Rcat": Rcat,
                        "R8": R8})
    return in_maps


def gather_outputs(per_core_results):
    outs = [per_core_results[c]["out"] for c in range(NCORES)]
    return np.concatenate(outs, axis=0)


def kernel_run(x, W_z, W_r, W_h, R_z, R_r, R_h, B_z, B_r, B_h, t_steps=T, **run_kw):
    inputs = dict(x=x, W_z=W_z, W_r=W_r, W_h=W_h, R_z=R_z, R_r=R_r, R_h=R_h,
                  B_z=B_z, B_r=B_r, B_h=B_h)
    in_maps = prepare_in_maps(inputs, t_steps)
    res = run_bass_kernel_spmd(_get_nc(t_steps), in_maps, list(range(NCORES)), **run_kw)
    full = gather_outputs(res.results)
    return full, res


def kernel(**inputs):
    full, _ = kernel_run(**inputs)
    return full


# revision 21
# speedup vs baseline: 1.1226x; 1.0768x over previous
"""GRU kernel for Trainium2, 8 NeuronCores, data-parallel over batch.

Reference computation (per timestep, batch-major):
    z = sigmoid(x_t @ W_z + s @ R_z + B_z)
    r = sigmoid(x_t @ W_r + s @ R_r + B_r)
    h = tanh   (x_t @ W_h + (r*s) @ R_h + B_h)
    s = (1-z)*s + z*h
Returns final s: [B, H].

Shapes: B=128, T=1024, D=512, H=1024.  Sharding: batch 16 per core.

Kernel design (per core):
  Phase A: XP = x @ [W_r|W_z|W_h] + B  precomputed for all timesteps at full
           PE efficiency (M=128 tiles), stored to internal DRAM [BC*T, 3H].
  Phase B: sequential scan, state kept ONLY in transposed layout sT [H-chunks
           on partitions, batch on free]. Per step:
           - ps_{r,z,h} [16,H] PSUM seeded with XP[t] via identity matmuls
           - r/z recurrent matmuls in fp8 (e4m3) DoubleRow perf mode: both
             R_{r,z} (resident, pre-quantized on host) and the state sT8
             (quantized each step) are fp8; 2 K-chunks per instruction.
           - h-gate uses rs8 = fp8(rT (.) sT) with the weight residual
             R_h ~ Rh8 + dRh8 (two DR passes) compensating fp8 quantization.
           - sigmoid/tanh on ScalarE (batch-major); r/z/h transposed back via
             PE transposes (z early, evacuated to SBUF mid-step); transposed
             update sT8 = fp8(sT + zT (.) (hT - sT)) fused on VectorE.
           Numerics (numpy emulation, full 1024 steps): rel err 1.26e-2 vs
           f32 reference; HW measured 1.257e-2 (< 2e-2 gate).
"""

import numpy as np

import concourse.bass as bass
from concourse import bacc
import concourse.mybir as mybir
from concourse.tile import TileContext
from concourse.bass_utils import run_bass_kernel_spmd
from concourse.masks import make_identity

B, T, D, H = 128, 1024, 512, 1024
NCORES = 8
BC = B // NCORES          # 16 batch rows per core
H3 = 3 * H                # gates concatenated [r|z|h]
KD = D // 128             # 4 k-chunks over input features
KH = H // 128             # 8 k-chunks over hidden dim
FP = mybir.dt.float32
FPR = mybir.dt.float32r
F8 = mybir.dt.float8e4
AF = mybir.ActivationFunctionType
OP = mybir.AluOpType
BF = mybir.dt.bfloat16
DR = mybir.MatmulPerfMode.DoubleRow
USE_DR_ZR = True     # fp8 DoubleRow for z/r gates
USE_DR_H = True      # fp8 DoubleRow (+residual) for h gate
USE_ZT_DMA = False   # DMA XBAR z-transpose races on HW; use PE


def _r(ap):
    # Bitcast an f32 AP to float32r (fast PE streaming, 1 cycle/row at N>=256)
    return ap.bitcast(FPR)


def build_gru(t_steps=T):
    nc = bacc.Bacc()
    xT = nc.declare_dram_parameter("xT", [D, BC * t_steps], FPR, False)
    Wc = nc.declare_dram_parameter("Wcat", [D, H3], FPR, False)
    Bc = nc.declare_dram_parameter("Bcat", [128, H3], FP, False)
    Rc = nc.declare_dram_parameter("Rcat", [H, H3], FPR, False)
    R8 = nc.declare_dram_parameter("R8", [H, 4 * H], F8, False)
    out = nc.declare_dram_parameter("out", [BC, H], FP, True)
    XP = nc.dram_tensor("XP", [BC * t_steps, H3], FPR)

    MT = (BC * t_steps) // 128   # number of 128-row tiles of [bt, .]
    NT = H3 // 512               # 6 n-tiles of 512
    xp3 = XP[:].rearrange("(b t) n -> t b n", b=BC)   # [t_steps, BC, H3]

    with TileContext(nc) as tc:
        with tc.tile_pool(name="const_pool", bufs=1) as cp:
            ident_t = cp.tile([16, 16], FP)
            make_identity(nc, ident_t[:])
            ident = cp.tile([16, 16], FPR)
            nc.scalar.copy(out=ident[:], in_=ident_t[:])
            ident128 = cp.tile([128, 128], FP)
            make_identity(nc, ident128[:])

            # ---------------- phase A: XP = x @ Wcat + B ----------------
            with (
                tc.tile_pool(name="phase_a_w", bufs=1) as wp,
                tc.tile_pool(name="a_x", bufs=4) as axp,
                tc.tile_pool(name="a_ps", bufs=4, space="PSUM") as aps,
                tc.tile_pool(name="a_out", bufs=4) as aop,
            ):
                # bias arrives pre-broadcast over 128 partitions from the host
                bias_bc = wp.tile([128, H3], FP)
                nc.sync.dma_start(out=bias_bc[:], in_=Bc[:, :])

                w_sb = wp.tile([128, KD * H3], FPR)
                nc.sync.dma_start(
                    out=w_sb[:],
                    in_=Wc[:].rearrange("(kd p) n -> p kd n", kd=KD),
                )
                xT_v = xT[:].rearrange("(kd p) m -> p kd m", kd=KD)
                for mt in range(MT):
                    x_sb = axp.tile([128, KD * 128], FPR)
                    nc.sync.dma_start(
                        out=x_sb[:],
                        in_=xT_v[:, :, mt * 128:(mt + 1) * 128],
                    )
                    for ntile in range(NT):
                        ps = aps.tile([128, 512], FP, tag="a_ps")
                        for kd in range(KD):
                            nc.tensor.matmul(
                                ps[:],
                                x_sb[:, kd * 128:(kd + 1) * 128],
                                w_sb[:, kd * H3 + ntile * 512: kd * H3 + (ntile + 1) * 512],
                                start=(kd == 0),
                                stop=(kd == KD - 1),
                            )
                        o_sb = aop.tile([128, 512], FPR)
                        nc.vector.tensor_tensor(
                            o_sb[:], ps[:], bias_bc[:, ntile * 512:(ntile + 1) * 512],
                            OP.add,
                        )
                        nc.sync.dma_start(
                            out=XP[mt * 128:(mt + 1) * 128,
                                   ntile * 512:(ntile + 1) * 512],
                            in_=o_sb[:],
                        )

            # ---------------- phase B: the scan ----------------
            with (
                tc.tile_pool(name="scan_state", bufs=1) as stp,
                tc.tile_pool(name="xp_in", bufs=3) as xpp,
                tc.tile_pool(name="gate_ps", bufs=1, space="PSUM") as gpp,
                tc.tile_pool(name="tr_ps", bufs=2, space="PSUM") as trp,
                tc.tile_pool(name="ew", bufs=2) as ewp,
            ):
                scan_body(nc, tc, stp, xpp, gpp, trp, ewp, ident, ident_t,
                          ident128, Rc, R8, XP, xp3, out, t_steps)
    nc.finalize()
    return nc


def scan_body(nc, tc, stp, xpp, gpp, trp, ewp, ident, ident_t, ident128,
              Rc, R8, XP, xp3, out, t_steps):
    # resident recurrent weights, all fp8: [Rr8|Rz8|Rh8|dRh8] (32KB/part)
    R8_sb = stp.tile([128, KH * 4 * H], F8)
    nc.sync.dma_start(
        out=R8_sb[:],
        in_=R8[:].rearrange("(kh p) n -> p kh n", kh=KH),
    )
    R8_v = R8_sb[:].rearrange("p (kh n) -> p kh n", kh=KH)
    Rf_v = None
    if not (USE_DR_ZR and USE_DR_H):
        Rf_sb = stp.tile([128, KH * H3], FPR)
        nc.sync.dma_start(
            out=Rf_sb[:],
            in_=Rc[:].rearrange("(kh p) n -> p kh n", kh=KH),
        )
        Rf_v = Rf_sb[:].rearrange("p (kh n) -> p kh n", kh=KH)

    # State lives ONLY transposed, split in two half tiles (kh 0-3 / 4-7) so
    # every consumer dependency is half-granular: the tail of step t overlaps
    # the head of step t+1.
    KH2 = KH // 2
    HW_ = KH2 * 16                      # 64 free elements per half
    sT = [stp.tile([128, HW_], FP, name=f"sT{i}") for i in range(2)]
    sT8 = [stp.tile([128, HW_], F8, name=f"sT8_{i}") for i in range(2)]
    for hf in range(2):
        nc.gpsimd.memset(sT[hf][:], 0.0)
        nc.gpsimd.memset(sT8[hf][:].bitcast(FP), 0.0)

    def sT8_pair(kp):
        # stationary AP [128, 2, 16] for k-chunk pair (2kp, 2kp+1)
        hf, j = divmod(kp, 2)
        return sT8[hf][:].rearrange("p (k m) -> p k m", k=KH2)[:, 2 * j:2 * j + 2, :]

    def inject(ps, xp, gate, ntile):
        lo = ntile * 512
        nc.tensor.matmul(
            ps[:], ident[:], xp[:, gate * H + lo: gate * H + lo + 512],
            start=True, stop=False,
        )

    def dr_matmuls(ps, gate, ntile):
        """ps[16,512] += sT8.T @ R8[:, gate-ntile] via fp8 DoubleRow."""
        lo = gate * H + ntile * 512
        if USE_DR_ZR:
            for kp in range(KH // 2):
                nc.tensor.matmul(
                    ps[:], sT8_pair(kp),
                    R8_v[:, 2 * kp:2 * kp + 2, lo:lo + 512],
                    start=False, stop=(kp == KH // 2 - 1),
                    perf_mode=DR,
                )
        else:
            for kh in range(KH):
                hf, j = divmod(kh, KH2)
                nc.tensor.matmul(
                    ps[:], _r(sT[hf][:, j * 16:j * 16 + 16]),
                    Rf_v[:, kh, lo:lo + 512],
                    start=False, stop=(kh == KH - 1),
                )

    def h_matmuls(ps, rs8, ntile):
        if USE_DR_H:
            # ps += rs8.T @ (Rh8 + dRh8), both fp8 DoubleRow passes
            for blk in (2, 3):
                lo = blk * H + ntile * 512
                for kp in range(KH // 2):
                    hf, j = divmod(kp, 2)
                    pair = rs8[hf][:].rearrange("p (k m) -> p k m", k=KH2)[:, 2 * j:2 * j + 2, :]
                    nc.tensor.matmul(
                        ps[:], pair,
                        R8_v[:, 2 * kp:2 * kp + 2, lo:lo + 512],
                        start=False, stop=(blk == 3 and kp == KH // 2 - 1),
                        perf_mode=DR,
                    )
        else:
            lo = 2 * H + ntile * 512
            for kh in range(KH):
                hf, j = divmod(kh, KH2)
                nc.tensor.matmul(
                    ps[:], rs8[hf][:, j * 16:j * 16 + 16],
                    Rf_v[:, kh, lo:lo + 512],
                    start=False, stop=(kh == KH - 1),
                )

    def transpose_half(dst_ps, src_sb, hf):
        # 4 chunk transposes of src half hf into a [128, 64] psum tile
        for j in range(KH2):
            kh = hf * KH2 + j
            nc.tensor.transpose(
                dst_ps[:, j * 16:(j + 1) * 16],
                src_sb[:, kh * 128:(kh + 1) * 128], ident_t[:]
            )

    def alloc_ps(t):
        _ = t
        return {g: [gpp.tile([16, 512], FP, tag=f"ps_{g}{b}", name=f"ps_{g}{b}_{t}")
                    for b in range(2)]
                for g in "rzh"}

    def alloc_xp(t):
        xp = xpp.tile([16, H3], FPR, tag="xp")
        nc.sync.dma_start(out=xp[:], in_=xp3[t])
        return xp

    xp = alloc_xp(0)
    ps = alloc_ps(0)
    for g, gi in (("r", 0), ("z", 1), ("h", 2)):
        for b in range(2):
            inject(ps[g][b], xp, gi, b)

    for t in range(t_steps):
        last = t + 1 >= t_steps
        # ---- r/z gates: fp8 DR matmuls ----
        for b in range(2):
            dr_matmuls(ps["r"][b], 0, b)
        for b in range(2):
            dr_matmuls(ps["z"][b], 1, b)
        if not last:
            nxp = alloc_xp(t + 1)
            nps = alloc_ps(t + 1)

        r_sb = ewp.tile([16, H], FP, tag="r")
        nc.scalar.activation(r_sb[:, 0:512], ps["r"][0][:], AF.Sigmoid)
        nc.scalar.activation(r_sb[:, 512:], ps["r"][1][:], AF.Sigmoid)

        # transpose psum tiles share one 2-buf rotation (2 banks total); the
        # z transposes are emitted LAST so their bank reuse never cycles.
        tps_r = [trp.tile([128, HW_], FP, tag="tr", name=f"tps_r{hf}_{t}")
                 for hf in range(2)]
        rs8 = [ewp.tile([128, HW_], F8 if USE_DR_H else FPR,
                        tag=f"rs8{hf}", name=f"rs8{hf}_{t}")
               for hf in range(2)]
        for hf in range(2):
            transpose_half(tps_r[hf], r_sb, hf)
        if not last:     # PE filler between r-transposes and h-matmuls
            for b in range(2):
                inject(nps["r"][b], nxp, 0, b)
        for hf in range(2):
            nc.vector.tensor_tensor(rs8[hf][:], tps_r[hf][:], sT[hf][:],
                                    OP.mult)

        # ---- h gate: fp8 DR matmuls with stationary rs8 ----
        h_matmuls(ps["h"][0], rs8, 0)
        h_matmuls(ps["h"][1], rs8, 1)

        # z: sigmoid, PE-transpose mid-step (PE bubbles during h matmuls),
        # then evacuate to SBUF early so the tail never waits on PE for z.
        z_sb = ewp.tile([16, H], FP, tag="z")
        nc.scalar.activation(z_sb[:, 0:512], ps["z"][0][:], AF.Sigmoid)
        nc.scalar.activation(z_sb[:, 512:], ps["z"][1][:], AF.Sigmoid)
        tps_z = [trp.tile([128, HW_], FP, tag="tr", name=f"tps_z{hf}_{t}")
                 for hf in range(2)]
        zTs = [ewp.tile([128, HW_], FP, tag=f"zTs{hf}", name=f"zTs{hf}_{t}")
               for hf in range(2)]
        for hf in range(2):
            transpose_half(tps_z[hf], z_sb, hf)
            nc.vector.tensor_copy(zTs[hf][:], tps_z[hf][:])
        zT_src = [zTs[hf][:] for hf in range(2)]
        if not last:
            for b in range(2):
                inject(nps["z"][b], nxp, 1, b)

        # ---- tanh + h transposes ----
        h_sb = ewp.tile([16, H], FP, tag="h")
        tps_h = [trp.tile([128, HW_], FP, tag="tr", name=f"tps_h{hf}_{t}")
                 for hf in range(2)]
        for hf in range(2):
            sl = slice(hf * 512, (hf + 1) * 512)
            nc.scalar.activation(h_sb[:, sl], ps["h"][hf][:], AF.Tanh)
            transpose_half(tps_h[hf], h_sb, hf)
        if not last:
            for hf in range(2):
                inject(nps["h"][hf], nxp, 2, hf)

        # ---- transposed update: sT += zT * (hT - sT);  sT8 = fp8(sT) ----
        for hf in range(2):
            dT = ewp.tile([128, HW_], FP, tag=f"dT{hf}", name=f"dT{hf}_{t}")
            nc.vector.tensor_tensor(dT[:], tps_h[hf][:], sT[hf][:], OP.subtract)
            zdT = ewp.tile([128, HW_], FP, tag=f"zdT{hf}", name=f"zdT{hf}_{t}")
            nc.vector.tensor_tensor(zdT[:], zT_src[hf], dT[:], OP.mult)
            nc.vector.tensor_tensor(sT8[hf][:], sT[hf][:], zdT[:], OP.add)
            nc.vector.tensor_tensor(_r(sT[hf][:]), sT[hf][:], zdT[:], OP.add)

        if not last:
            xp, ps = nxp, nps

    # transpose the final state back to batch-major and store
    po = [gpp.tile([16, 512], FP, tag=f"ps_r{b}", name=f"po{b}") for b in range(2)]
    for kh in range(KH):
        hf, j = divmod(kh, KH2)
        b, jj = divmod(kh, KH2)
        nc.tensor.transpose(
            po[b][:, jj * 128:(jj + 1) * 128],
            sT[hf][:, j * 16:(j + 1) * 16], ident128[:]
        )
    s_out = ewp.tile([16, H], FP, tag="r")
    nc.scalar.copy(out=s_out[:, 0:512], in_=po[0][:])
    nc.scalar.copy(out=s_out[:, 512:], in_=po[1][:])
    nc.sync.dma_start(out=out[:, :], in_=s_out[:])


_CACHE = {}


def _get_nc(t_steps=T):
    key = t_steps
    if key not in _CACHE:
        _CACHE[key] = build_gru(t_steps)
    return _CACHE[key]


def prepare_in_maps(inputs, t_steps=T):
    f8np = mybir.dt.np(F8)
    x = np.asarray(inputs["x"], dtype=np.float32)
    Wcat = np.ascontiguousarray(
        np.concatenate([np.asarray(inputs["W_r"]), np.asarray(inputs["W_z"]),
                        np.asarray(inputs["W_h"])], axis=1),
        dtype=np.float32,
    )
    Rcat = np.ascontiguousarray(
        np.concatenate([np.asarray(inputs["R_r"]), np.asarray(inputs["R_z"]),
                        np.asarray(inputs["R_h"])], axis=1),
        dtype=np.float32,
    )
    Rh8 = Rcat[:, 2 * H:].astype(f8np)
    dRh8 = (Rcat[:, 2 * H:] - Rh8.astype(np.float32)).astype(f8np)
    R8 = np.ascontiguousarray(
        np.concatenate([Rcat[:, :2 * H].astype(f8np), Rh8, dRh8], axis=1))
    Bcat = np.ascontiguousarray(
        np.broadcast_to(
            np.concatenate([np.asarray(inputs["B_r"]), np.asarray(inputs["B_z"]),
                            np.asarray(inputs["B_h"])])[None, :], (128, H3)),
        dtype=np.float32,
    )
    in_maps = []
    for c in range(NCORES):
        xc = x[c * BC:(c + 1) * BC, :t_steps, :]          # [BC, t, D]
        xTc = np.ascontiguousarray(
            xc.transpose(2, 0, 1).reshape(D, BC * t_steps)
        )
        in_maps.append({"xT": xTc, "Wcat": Wcat, "Bcat": Bcat, "Rcat": Rcat,
                        "R8": R8})
    return in_maps


def gather_outputs(per_core_results):
    outs = [per_core_results[c]["out"] for c in range(NCORES)]
    return np.concatenate(outs, axis=0)


def kernel_run(x, W_z, W_r, W_h, R_z, R_r, R_h, B_z, B_r, B_h, t_steps=T, **run_kw):
    inputs = dict(x=x, W_z=W_z, W_r=W_r, W_h=W_h, R_z=R_z, R_r=R_r, R_h=R_h,
                  B_z=B_z, B_r=B_r, B_h=B_h)
    in_maps = prepare_in_maps(inputs, t_steps)
    res = run_bass_kernel_spmd(_get_nc(t_steps), in_maps, list(range(NCORES)), **run_kw)
    full = gather_outputs(res.results)
    return full, res


def kernel(**inputs):
    full, _ = kernel_run(**inputs)
    return full


# revision 33
# speedup vs baseline: 1.2044x; 1.0729x over previous
"""GRU kernel for Trainium2, 8 NeuronCores, data-parallel over batch.

Reference computation (per timestep, batch-major):
    z = sigmoid(x_t @ W_z + s @ R_z + B_z)
    r = sigmoid(x_t @ W_r + s @ R_r + B_r)
    h = tanh   (x_t @ W_h + (r*s) @ R_h + B_h)
    s = (1-z)*s + z*h
Returns final s: [B, H].

Shapes: B=128, T=1024, D=512, H=1024.  Sharding: batch 16 per core.

Kernel design (per core):
  Phase A: XP = x @ [W_r|W_z|W_h] + B  precomputed for all timesteps at full
           PE efficiency (M=128 tiles), stored to internal DRAM [BC*T, 3H].
  Phase B: sequential scan, state kept ONLY in transposed layout sT [H-chunks
           on partitions, batch on free]. Per step:
           - ps_{r,z,h} [16,H] PSUM seeded with XP[t] via identity matmuls
           - r/z recurrent matmuls in fp8 (e4m3) DoubleRow perf mode: both
             R_{r,z} (resident, pre-quantized on host) and the state sT8
             (quantized each step) are fp8; 2 K-chunks per instruction.
           - h-gate uses rs8 = fp8(rT (.) sT) with the weight residual
             R_h ~ Rh8 + dRh8 (two DR passes) compensating fp8 quantization.
           - sigmoid/tanh on ScalarE (batch-major); r/z/h transposed back via
             PE transposes (z early, evacuated to SBUF mid-step); transposed
             update sT8 = fp8(sT + zT (.) (hT - sT)) fused on VectorE.
           Numerics (numpy emulation, full 1024 steps): rel err 1.26e-2 vs
           f32 reference; HW measured 1.257e-2 (< 2e-2 gate).
"""

import numpy as np

import concourse.bass as bass
from concourse import bacc
import concourse.mybir as mybir
from concourse.tile import TileContext
from concourse.bass_utils import run_bass_kernel_spmd
from concourse.masks import make_identity

B, T, D, H = 128, 1024, 512, 1024
NCORES = 8
BC = B // NCORES          # 16 batch rows per core
H3 = 3 * H                # gates concatenated [r|z|h]
KD = D // 128             # 4 k-chunks over input features
KH = H // 128             # 8 k-chunks over hidden dim
FP = mybir.dt.float32
FPR = mybir.dt.float32r
F8 = mybir.dt.float8e4
AF = mybir.ActivationFunctionType
OP = mybir.AluOpType
BF = mybir.dt.bfloat16
DR = mybir.MatmulPerfMode.DoubleRow
USE_DR_ZR = True     # fp8 DoubleRow for z/r gates
USE_DR_H = True      # fp8 DoubleRow (+residual) for h gate
USE_ZT_DMA = False   # DMA XBAR z-transpose races on HW; use PE


def _r(ap):
    # Bitcast an f32 AP to float32r (fast PE streaming, 1 cycle/row at N>=256)
    return ap.bitcast(FPR)


def build_gru(t_steps=T):
    nc = bacc.Bacc()
    xT = nc.declare_dram_parameter("xT", [D, BC * t_steps], FPR, False)
    Wc = nc.declare_dram_parameter("Wcat", [D, H3], FPR, False)
    Bc = nc.declare_dram_parameter("Bcat", [128, H3], FP, False)
    Rc = nc.declare_dram_parameter("Rcat", [H, H3], FPR, False)
    R8 = nc.declare_dram_parameter("R8", [H, 4 * H], F8, False)
    out = nc.declare_dram_parameter("out", [BC, H], FP, True)
    XP = nc.dram_tensor("XP", [BC * t_steps, H3], BF)

    MT = (BC * t_steps) // 128   # number of 128-row tiles of [bt, .]
    NT = H3 // 512               # 6 n-tiles of 512
    xp3 = XP[:].rearrange("(b t) n -> t b n", b=BC)   # [t_steps, BC, H3]

    with TileContext(nc) as tc:
        with tc.tile_pool(name="const_pool", bufs=1) as cp:
            ident_t = cp.tile([16, 16], FP)
            make_identity(nc, ident_t[:])
            ident = cp.tile([16, 16], BF)
            nc.scalar.copy(out=ident[:], in_=ident_t[:])
            ident128 = cp.tile([128, 128], FP)
            make_identity(nc, ident128[:])

            # ---------------- phase A: XP = x @ Wcat + B ----------------
            with (
                tc.tile_pool(name="phase_a_w", bufs=1) as wp,
                tc.tile_pool(name="a_x", bufs=4) as axp,
                tc.tile_pool(name="a_ps", bufs=4, space="PSUM") as aps,
                tc.tile_pool(name="a_out", bufs=4) as aop,
            ):
                # bias arrives pre-broadcast over 128 partitions from the host
                bias_bc = wp.tile([128, H3], FP)
                nc.sync.dma_start(out=bias_bc[:], in_=Bc[:, :])

                w_sb = wp.tile([128, KD * H3], FPR)
                nc.sync.dma_start(
                    out=w_sb[:],
                    in_=Wc[:].rearrange("(kd p) n -> p kd n", kd=KD),
                )
                xT_v = xT[:].rearrange("(kd p) m -> p kd m", kd=KD)
                for mt in range(MT):
                    x_sb = axp.tile([128, KD * 128], FPR)
                    nc.sync.dma_start(
                        out=x_sb[:],
                        in_=xT_v[:, :, mt * 128:(mt + 1) * 128],
                    )
                    for ntile in range(NT):
                        ps = aps.tile([128, 512], FP, tag="a_ps")
                        for kd in range(KD):
                            nc.tensor.matmul(
                                ps[:],
                                x_sb[:, kd * 128:(kd + 1) * 128],
                                w_sb[:, kd * H3 + ntile * 512: kd * H3 + (ntile + 1) * 512],
                                start=(kd == 0),
                                stop=(kd == KD - 1),
                            )
                        o_sb = aop.tile([128, 512], BF)
                        nc.vector.tensor_tensor(
                            o_sb[:], ps[:], bias_bc[:, ntile * 512:(ntile + 1) * 512],
                            OP.add,
                        )
                        nc.sync.dma_start(
                            out=XP[mt * 128:(mt + 1) * 128,
                                   ntile * 512:(ntile + 1) * 512],
                            in_=o_sb[:],
                        )

            # ---------------- phase B: the scan ----------------
            with (
                tc.tile_pool(name="scan_state", bufs=1) as stp,
                tc.tile_pool(name="xp_in", bufs=3) as xpp,
                tc.tile_pool(name="gate_ps", bufs=1, space="PSUM") as gpp,
                tc.tile_pool(name="tr_ps", bufs=2, space="PSUM") as trp,
                tc.tile_pool(name="ew", bufs=2) as ewp,
            ):
                scan_body(nc, tc, stp, xpp, gpp, trp, ewp, ident, ident_t,
                          ident128, Rc, R8, XP, xp3, out, t_steps)
    nc.finalize()
    return nc


def scan_body(nc, tc, stp, xpp, gpp, trp, ewp, ident, ident_t, ident128,
              Rc, R8, XP, xp3, out, t_steps):
    # resident recurrent weights, all fp8: [Rr8|Rz8|Rh8|dRh8] (32KB/part)
    R8_sb = stp.tile([128, KH * 4 * H], F8)
    nc.sync.dma_start(
        out=R8_sb[:],
        in_=R8[:].rearrange("(kh p) n -> p kh n", kh=KH),
    )
    R8_v = R8_sb[:].rearrange("p (kh n) -> p kh n", kh=KH)
    Rf_v = None
    if not (USE_DR_ZR and USE_DR_H):
        Rf_sb = stp.tile([128, KH * H3], FPR)
        nc.sync.dma_start(
            out=Rf_sb[:],
            in_=Rc[:].rearrange("(kh p) n -> p kh n", kh=KH),
        )
        Rf_v = Rf_sb[:].rearrange("p (kh n) -> p kh n", kh=KH)

    # State lives ONLY transposed, split in two half tiles (kh 0-3 / 4-7) so
    # every consumer dependency is half-granular: the tail of step t overlaps
    # the head of step t+1.
    KH2 = KH // 2
    HW_ = KH2 * 16                      # 64 free elements per half
    sT = [stp.tile([128, HW_], FP, name=f"sT{i}") for i in range(2)]
    sT8 = [stp.tile([128, HW_], F8, name=f"sT8_{i}") for i in range(2)]
    for hf in range(2):
        nc.gpsimd.memset(sT[hf][:], 0.0)
        nc.gpsimd.memset(sT8[hf][:].bitcast(FP), 0.0)

    def sT8_pair(kp):
        # stationary AP [128, 2, 16] for k-chunk pair (2kp, 2kp+1)
        hf, j = divmod(kp, 2)
        return sT8[hf][:].rearrange("p (k m) -> p k m", k=KH2)[:, 2 * j:2 * j + 2, :]

    def inject(ps, xp, gate, ntile):
        lo = ntile * 512
        nc.tensor.matmul(
            ps[:], ident[:], xp[:, gate * H + lo: gate * H + lo + 512],
            start=True, stop=False,
        )

    def dr_matmuls(ps, gate, ntile):
        """ps[16,512] += sT8.T @ R8[:, gate-ntile] via fp8 DoubleRow."""
        lo = gate * H + ntile * 512
        if USE_DR_ZR:
            for kp in range(KH // 2):
                nc.tensor.matmul(
                    ps[:], sT8_pair(kp),
                    R8_v[:, 2 * kp:2 * kp + 2, lo:lo + 512],
                    start=False, stop=(kp == KH // 2 - 1),
                    perf_mode=DR,
                )
        else:
            for kh in range(KH):
                hf, j = divmod(kh, KH2)
                nc.tensor.matmul(
                    ps[:], _r(sT[hf][:, j * 16:j * 16 + 16]),
                    Rf_v[:, kh, lo:lo + 512],
                    start=False, stop=(kh == KH - 1),
                )

    def h_matmuls(ps, rs8, ntile):
        if USE_DR_H:
            # ps += rs8.T @ (Rh8 + dRh8), both fp8 DoubleRow passes
            for blk in (2, 3):
                lo = blk * H + ntile * 512
                for kp in range(KH // 2):
                    hf, j = divmod(kp, 2)
                    pair = rs8[hf][:].rearrange("p (k m) -> p k m", k=KH2)[:, 2 * j:2 * j + 2, :]
                    nc.tensor.matmul(
                        ps[:], pair,
                        R8_v[:, 2 * kp:2 * kp + 2, lo:lo + 512],
                        start=False, stop=(blk == 3 and kp == KH // 2 - 1),
                        perf_mode=DR,
                    )
        else:
            lo = 2 * H + ntile * 512
            for kh in range(KH):
                hf, j = divmod(kh, KH2)
                nc.tensor.matmul(
                    ps[:], rs8[hf][:, j * 16:j * 16 + 16],
                    Rf_v[:, kh, lo:lo + 512],
                    start=False, stop=(kh == KH - 1),
                )

    def transpose_half(dst_ps, src_sb, hf):
        # 4 chunk transposes of src half hf into a [128, 64] psum tile
        for j in range(KH2):
            kh = hf * KH2 + j
            nc.tensor.transpose(
                dst_ps[:, j * 16:(j + 1) * 16],
                src_sb[:, kh * 128:(kh + 1) * 128], ident_t[:]
            )

    def alloc_ps(t):
        _ = t
        return {g: [gpp.tile([16, 512], FP, tag=f"ps_{g}{b}", name=f"ps_{g}{b}_{t}")
                    for b in range(2)]
                for g in "rzh"}

    def alloc_xp(t):
        xp = xpp.tile([16, H3], BF, tag="xp")
        nc.sync.dma_start(out=xp[:], in_=xp3[t])
        return xp

    xp = alloc_xp(0)
    ps = alloc_ps(0)
    for g, gi in (("r", 0), ("z", 1), ("h", 2)):
        for b in range(2):
            inject(ps[g][b], xp, gi, b)

    for t in range(t_steps):
        last = t + 1 >= t_steps
        # ---- r/z gates: fp8 DR matmuls ----
        for b in range(2):
            dr_matmuls(ps["r"][b], 0, b)
        for b in range(2):
            dr_matmuls(ps["z"][b], 1, b)
        if not last:
            nxp = alloc_xp(t + 1)
            nps = alloc_ps(t + 1)

        r_sb = ewp.tile([16, H], FP, tag="r")
        nc.scalar.activation(r_sb[:, 0:512], ps["r"][0][:], AF.Sigmoid)
        nc.scalar.activation(r_sb[:, 512:], ps["r"][1][:], AF.Sigmoid)

        # transpose psum tiles share one 2-buf rotation (2 banks total); the
        # z transposes are emitted LAST so their bank reuse never cycles.
        tps_r = [trp.tile([128, HW_], FP, tag="tr", name=f"tps_r{hf}_{t}")
                 for hf in range(2)]
        rs8 = [ewp.tile([128, HW_], F8 if USE_DR_H else FPR,
                        tag=f"rs8{hf}", name=f"rs8{hf}_{t}")
               for hf in range(2)]
        for hf in range(2):
            transpose_half(tps_r[hf], r_sb, hf)
        if not last:     # PE filler between r-transposes and h-matmuls
            for b in range(2):
                inject(nps["r"][b], nxp, 0, b)
        for hf in range(2):
            nc.vector.tensor_tensor(rs8[hf][:], tps_r[hf][:], sT[hf][:],
                                    OP.mult)

        # ---- h gate: fp8 DR matmuls with stationary rs8 ----
        h_matmuls(ps["h"][0], rs8, 0)
        h_matmuls(ps["h"][1], rs8, 1)

        # z: sigmoid, PE-transpose mid-step (PE bubbles during h matmuls),
        # then evacuate to SBUF early so the tail never waits on PE for z.
        z_sb = ewp.tile([16, H], FP, tag="z")
        nc.scalar.activation(z_sb[:, 0:512], ps["z"][0][:], AF.Sigmoid)
        nc.scalar.activation(z_sb[:, 512:], ps["z"][1][:], AF.Sigmoid)
        tps_z = [trp.tile([128, HW_], FP, tag="tr", name=f"tps_z{hf}_{t}")
                 for hf in range(2)]
        zTs = [ewp.tile([128, HW_], FP, tag=f"zTs{hf}", name=f"zTs{hf}_{t}")
               for hf in range(2)]
        for hf in range(2):
            transpose_half(tps_z[hf], z_sb, hf)
            nc.vector.tensor_copy(zTs[hf][:], tps_z[hf][:])
        zT_src = [zTs[hf][:] for hf in range(2)]
        if not last:
            for b in range(2):
                inject(nps["z"][b], nxp, 1, b)

        # ---- tanh + h transposes ----
        h_sb = ewp.tile([16, H], FP, tag="h")
        tps_h = [trp.tile([128, HW_], FP, tag="tr", name=f"tps_h{hf}_{t}")
                 for hf in range(2)]
        for hf in range(2):
            sl = slice(hf * 512, (hf + 1) * 512)
            nc.scalar.activation(h_sb[:, sl], ps["h"][hf][:], AF.Tanh)
            transpose_half(tps_h[hf], h_sb, hf)
        if not last:
            for hf in range(2):
                inject(nps["h"][hf], nxp, 2, hf)

        # ---- transposed update: sT += zT * (hT - sT);  sT8 = fp8(sT) ----
        for hf in range(2):
            dT = ewp.tile([128, HW_], FP, tag=f"dT{hf}", name=f"dT{hf}_{t}")
            nc.vector.tensor_tensor(dT[:], tps_h[hf][:], sT[hf][:], OP.subtract)
            zdT = ewp.tile([128, HW_], FP, tag=f"zdT{hf}", name=f"zdT{hf}_{t}")
            nc.vector.tensor_tensor(zdT[:], zT_src[hf], dT[:], OP.mult)
            nc.vector.tensor_tensor(sT8[hf][:], sT[hf][:], zdT[:], OP.add)
            nc.vector.tensor_tensor(_r(sT[hf][:]), sT[hf][:], zdT[:], OP.add)

        if not last:
            xp, ps = nxp, nps

    # transpose the final state back to batch-major and store
    po = [gpp.tile([16, 512], FP, tag=f"ps_r{b}", name=f"po{b}") for b in range(2)]
    for kh in range(KH):
        hf, j = divmod(kh, KH2)
        b, jj = divmod(kh, KH2)
        nc.tensor.transpose(
            po[b][:, jj * 128:(jj + 1) * 128],
            sT[hf][:, j * 16:(j + 1) * 16], ident128[:]
        )
    s_out = ewp.tile([16, H], FP, tag="r")
    nc.scalar.copy(out=s_out[:, 0:512], in_=po[0][:])
    nc.scalar.copy(out=s_out[:, 512:], in_=po[1][:])
    nc.sync.dma_start(out=out[:, :], in_=s_out[:])


_CACHE = {}


def _get_nc(t_steps=T):
    key = t_steps
    if key not in _CACHE:
        _CACHE[key] = build_gru(t_steps)
    return _CACHE[key]


def prepare_in_maps(inputs, t_steps=T):
    f8np = mybir.dt.np(F8)
    x = np.asarray(inputs["x"], dtype=np.float32)
    Wcat = np.ascontiguousarray(
        np.concatenate([np.asarray(inputs["W_r"]), np.asarray(inputs["W_z"]),
                        np.asarray(inputs["W_h"])], axis=1),
        dtype=np.float32,
    )
    Rcat = np.ascontiguousarray(
        np.concatenate([np.asarray(inputs["R_r"]), np.asarray(inputs["R_z"]),
                        np.asarray(inputs["R_h"])], axis=1),
        dtype=np.float32,
    )
    Rh8 = Rcat[:, 2 * H:].astype(f8np)
    dRh8 = (Rcat[:, 2 * H:] - Rh8.astype(np.float32)).astype(f8np)
    R8 = np.ascontiguousarray(
        np.concatenate([Rcat[:, :2 * H].astype(f8np), Rh8, dRh8], axis=1))
    Bcat = np.ascontiguousarray(
        np.broadcast_to(
            np.concatenate([np.asarray(inputs["B_r"]), np.asarray(inputs["B_z"]),
                            np.asarray(inputs["B_h"])])[None, :], (128, H3)),
        dtype=np.float32,
    )
    in_maps = []
    for c in range(NCORES):
        xc = x[c * BC:(c + 1) * BC, :t_steps, :]          # [BC, t, D]
        xTc = np.ascontiguousarray(
            xc.transpose(2, 0, 1).reshape(D, BC * t_steps)
        )
        in_maps.append({"xT": xTc, "Wcat": Wcat, "Bcat": Bcat, "Rcat": Rcat,
                        "R8": R8})
    return in_maps


def gather_outputs(per_core_results):
    outs = [per_core_results[c]["out"] for c in range(NCORES)]
    return np.concatenate(outs, axis=0)


def kernel_run(x, W_z, W_r, W_h, R_z, R_r, R_h, B_z, B_r, B_h, t_steps=T, **run_kw):
    inputs = dict(x=x, W_z=W_z, W_r=W_r, W_h=W_h, R_z=R_z, R_r=R_r, R_h=R_h,
                  B_z=B_z, B_r=B_r, B_h=B_h)
    in_maps = prepare_in_maps(inputs, t_steps)
    res = run_bass_kernel_spmd(_get_nc(t_steps), in_maps, list(range(NCORES)), **run_kw)
    full = gather_outputs(res.results)
    return full, res


def kernel(**inputs):
    full, _ = kernel_run(**inputs)
    return full
